# revision 27
# baseline (speedup 1.0000x reference)
"""Encoder self-attention (RMSNorm + fused QKV + qk-norm + SDPA + scaled o_proj
+ residual) on 8 NeuronCores, data-parallel over the batch dim N=8.

v5: host folds the input RMSNorm into xT (qk-norm makes q/k scale-invariant;
the 1/16 fp8 weight scale and v normalization ride oT), fp8 DoubleRow QKV,
fine-grained AV interleave into the exp-paced score slots (keeps the PE dense
so the HAM clock stays warm), fused eviction+softmax-normalize on DVE,
Schraudolph fast-exp for 5/16 tiles, tail AV/o_proj interleave.
"""

import numpy as np
import ml_dtypes
from contextlib import ExitStack

import concourse.bass as bass
import concourse.mybir as mybir
import concourse.tile as tile
from concourse import bacc
from concourse.bass import ts
from concourse.masks import make_identity

F32 = mybir.dt.float32
F32R = mybir.dt.float32r
BF16 = mybir.dt.bfloat16
FP8 = mybir.dt.float8e4
I16 = mybir.dt.int16
AF = mybir.ActivationFunctionType
ALU = mybir.AluOpType
DR = mybir.MatmulPerfMode.DoubleRow

P = 128
D = 768
L = 1024
NH = 12
HD = 64
TQ = L // P      # 8 token tiles
KC = D // P      # 6 contraction chunks
CT = D // P      # 6 channel tiles (q/k each)
EPS = 1e-6
WSCALE = 16.0    # qkv weights are scaled by 16 on the host for fp8 range

# Schraudolph fast exp in bf16 domain: exp(x) ~= bitcast_bf16(int16(A16*x + B16))
A16 = float(2**7 / np.log(2))
B16 = float(127 * 2**7 - 366393.0 / 65536.0)

# (hh, jt) score tiles whose exp runs on the vector engine instead of scalar
DVE_EXP = {(0, 1), (1, 3), (0, 5), (1, 7), (1, 0)}


def build_bass():
    nc = bacc.Bacc(None, target_bir_lowering=False)

    x_d = nc.dram_tensor("x", [L, D], F32, kind="ExternalInput")
    xT_d = nc.dram_tensor("xT", [D, L], FP8, kind="ExternalInput")
    wT_d = nc.dram_tensor("wT", [D, 3 * D], FP8, kind="ExternalInput")
    oT_d = nc.dram_tensor("oT", [D, D], BF16, kind="ExternalInput")
    m_d = nc.dram_tensor("masks", [2, P], F32R, kind="ExternalInput")
    ones_d = nc.dram_tensor("ones", [P, NH], BF16, kind="ExternalInput")
    out_d = nc.dram_tensor("out", [L, D], F32, kind="ExternalOutput")

    with tile.TileContext(nc) as tc, ExitStack() as ctx:
        persist = ctx.enter_context(tc.tile_pool(name="persist", bufs=1))
        small = ctx.enter_context(tc.tile_pool(name="small", bufs=1))

        v_sb = [persist.tile([P, NH, HD + 1], BF16, tag=f"v{j}", name=f"v{j}")
                for j in range(TQ)]
        qnT = persist.tile([P, CT, L], BF16, tag="qnT", name="qnT")
        knT = persist.tile([P, CT, L], BF16, tag="knT", name="knT")
        x_keep = [persist.tile([P, D], F32, tag=f"x{i}", name=f"x{i}")
                  for i in range(TQ)]
        ident = small.tile([P, P], BF16, tag="ident")
        make_identity(nc, ident[:])
        eps_t = small.tile([P, 1], F32, tag="eps_t")
        nc.vector.memset(eps_t[:], EPS)
        eps64_t = small.tile([P, 1], F32, tag="eps64_t")
        nc.vector.memset(eps64_t[:], HD * EPS)

        # ================= phase A: QKV, qk-norm, transposes ================
        with (
            tc.tile_pool(name="wx", bufs=1) as wx,
            tc.tile_pool(name="qk_tmp", bufs=2) as qk_tmp,
            tc.tile_pool(name="scr", bufs=2) as scr,
            tc.tile_pool(name="psB", bufs=1, space="PSUM") as psB,
            tc.tile_pool(name="psT", bufs=2, space="PSUM") as psT,
        ):
            xT_all = wx.tile([P, KC, L], FP8, tag="xT_all", name="xT_all")
            wT_all = wx.tile([P, KC, 3 * D], FP8, tag="wT_all", name="wT_all")
            # DoubleRow consumes chunk pairs: land both chunks of a pair early
            for c0 in range(0, KC, 2):
                for c in (c0, c0 + 1):
                    nc.sync.dma_start(out=xT_all[:, c, :], in_=xT_d[ts(c, P), :])
                for c in (c0, c0 + 1):
                    nc.sync.dma_start(out=wT_all[:, c, :], in_=wT_d[ts(c, P), :])

            # warm the PE clock during the initial weight-DMA wait: scrap
            # transposes of the identity have no data dependencies and fill
            # the otherwise-idle launch window so QKV starts at full clock
            for _ in range(20):
                tp = psT.tile([P, 3 * P], BF16, tag="tp")
                for j in range(3):
                    nc.tensor.transpose(tp[:, ts(j, P)], ident[:], ident[:])

            tn_tiles = {}

            def emit_transposes(i):
                for src, dstT in ((tn_tiles[i][0], qnT), (tn_tiles[i][1], knT)):
                    for g in range(2):
                        tp = psT.tile([P, 3 * P], BF16, tag="tp")
                        for j in range(3):
                            nc.tensor.transpose(
                                tp[:, ts(j, P)], src[:, ts(3 * g + j, P)], ident[:]
                            )
                        if g == 0:
                            nc.vector.tensor_copy(
                                dstT[:, 0:3, ts(i, P)],
                                tp[:].rearrange("p (b q) -> p b q", q=P),
                            )
                        else:
                            nc.scalar.copy(
                                dstT[:, 3:6, ts(i, P)],
                                tp[:].rearrange("p (b q) -> p b q", q=P),
                            )

            for i in range(TQ):
                nc.sync.dma_start(out=x_keep[i][:], in_=x_d[ts(i, P), :])

                # QKV matmul: fp8 DoubleRow over 3 contraction pair-chunks
                ps = [psB.tile([P, 384], F32, tag=f"qkv{c}", name=f"qkvps{c}")
                      for c in range(6)]
                for dk in range(0, KC, 2):
                    lhsT = xT_all[:, dk : dk + 2, ts(i, P)]
                    for c in range(6):
                        nc.tensor.matmul(
                            ps[c][:],
                            lhsT,
                            wT_all[:, dk : dk + 2, ts(c, 384)],
                            start=(dk == 0),
                            stop=(dk == KC - 2),
                            perf_mode=DR,
                        )

                # evictions first: these free the QKV PSUM banks for the next
                # tile, so they must not queue behind the transpose evictions
                # (host already folded rstd into xT; the 16x weight scale
                # rides oT for v and cancels in qk-norm)
                q_t = qk_tmp.tile([P, D], BF16, tag="q_t")
                k_t = qk_tmp.tile([P, D], BF16, tag="k_t")
                for c in range(2):
                    nc.scalar.copy(q_t[:, ts(c, 384)], ps[c][:])
                    nc.scalar.copy(k_t[:, ts(c, 384)], ps[2 + c][:])
                    nc.vector.tensor_copy(
                        v_sb[i][:, 6 * c : 6 * c + 6, 0:HD],
                        ps[4 + c][:].rearrange("p (h d) -> p h d", d=HD),
                    )
                nc.sync.dma_start(
                    out=v_sb[i][:, :, HD : HD + 1], in_=ones_d[:, :, None]
                )

                # transposes of the previous tile (keeps PE dense)
                if i > 0:
                    emit_transposes(i - 1)

                # qk-norm (RMSNorm over each head's 64 channels)
                tns = []
                for t, isq in ((q_t, True), (k_t, False)):
                    sqg = scr.tile([P, D], F32, tag=f"sqg{int(isq)}")
                    if isq:
                        nc.gpsimd.tensor_mul(sqg[:], t[:], t[:])
                    else:
                        nc.scalar.activation(sqg[:], t[:], AF.Square)
                    ssg = scr.tile([P, NH, 1], F32, tag=f"ssg{int(isq)}")
                    nc.vector.tensor_reduce(
                        ssg[:, :, 0],
                        sqg[:].rearrange("p (h d) -> p h d", d=HD),
                        axis=mybir.AxisListType.X,
                        op=ALU.add,
                    )
                    # q also absorbs the 1/sqrt(hd) attention scale; the 16x
                    # weight scale cancels (scale-invariance of rms norm)
                    if isq:
                        nc.scalar.activation(ssg[:], ssg[:], AF.Sqrt, bias=eps64_t[:])
                    else:
                        nc.scalar.activation(
                            ssg[:], ssg[:], AF.Sqrt, scale=1.0 / HD, bias=eps_t[:]
                        )
                    rsg = scr.tile([P, NH, 1], F32, tag=f"rsg{int(isq)}")
                    nc.vector.reciprocal(rsg[:], ssg[:])
                    tn = scr.tile([P, D], BF16, tag=f"tn{int(isq)}")
                    nc.gpsimd.tensor_mul(
                        tn[:].rearrange("p (h d) -> p h d", d=HD),
                        t[:].rearrange("p (h d) -> p h d", d=HD),
                        rsg[:].to_broadcast((P, NH, HD)),
                    )
                    tns.append(tn)
                tn_tiles[i] = (tns[0], tns[1])
                last_qk = (q_t, k_t)

            # keep the PE clock warm through the last tile's norm chain:
            # scrap transposes that fire as its products materialize
            for src in (last_qk[0], last_qk[1], tn_tiles[TQ - 1][0],
                        tn_tiles[TQ - 1][1]):
                tp = psT.tile([P, 3 * P], BF16, tag="tp")
                for j in range(3):
                    nc.tensor.transpose(tp[:, ts(j, P)], src[:, ts(2 * j, P)],
                                        ident[:])
            emit_transposes(TQ - 1)

        # ======================= phase B/C: attention =======================
        with (
            tc.tile_pool(name="oT_pool", bufs=1) as oTp,
            tc.tile_pool(name="attnT_pool", bufs=1) as attnp,
            tc.tile_pool(name="expT", bufs=26) as expp,
            tc.tile_pool(name="expI", bufs=10) as expip,
            tc.tile_pool(name="rs", bufs=1) as rsp,
            tc.tile_pool(name="bcsb", bufs=2) as bcp,
            nc.allow_low_precision(reason="softmax denominators in fp32r"),
            tc.tile_pool(name="psC_av", bufs=3, space="PSUM") as ps_av,
            tc.tile_pool(name="psC_bc", bufs=1, space="PSUM") as ps_bc,
        ):
            oT_all = oTp.tile([P, CT, D], BF16, tag="oT_all", name="oT_all")
            nc.sync.dma_start(
                out=oT_all[:], in_=oT_d[:].rearrange("(c p) e -> p c e", p=P)
            )
            mask0 = rsp.tile([1, P], F32R, tag="m0")
            mask1 = rsp.tile([1, P], F32R, tag="m1")
            nc.sync.dma_start(out=mask0[:], in_=m_d[0:1, :])
            nc.sync.dma_start(out=mask1[:], in_=m_d[1:2, :])
            attnT = [attnp.tile([P, L], BF16, tag=f"attnT{c}", name=f"attnT{c}")
                     for c in range(CT)]

            exp_store = {}
            av_open = {}

            def emit_sc_exp(hp, jt, hh):
                off = HD * hh
                sc = ps_sc.tile([P, L], F32, tag="sc")
                for ic in range(2):
                    nc.tensor.matmul(
                        sc[:, ts(ic, 512)],
                        knT[off : off + HD, hp, ts(jt, P)],
                        qnT[off : off + HD, hp, ts(ic, 512)],
                        start=True,
                        stop=True,
                        tile_position=(off, 0),
                    )
                if (hh, jt) in DVE_EXP:
                    ei = expip.tile([P, L], I16, tag="eint",
                                    name=f"ei_{hp}_{hh}_{jt}")
                    nc.vector.tensor_scalar(
                        ei[:], sc[:], A16, B16, op0=ALU.mult, op1=ALU.add,
                    )
                    exp_store[(hp, hh, jt)] = ei[:].bitcast(BF16)
                else:
                    et = expp.tile([P, L], BF16, tag="exp",
                                   name=f"e_{hp}_{hh}_{jt}")
                    nc.scalar.activation(et[:], sc[:], AF.Exp)
                    exp_store[(hp, hh, jt)] = et[:]

            def emit_av_steps(hp, ic, jts):
                # accumulate both heads' AV for jt steps `jts` of (hp, ic)
                for jt in jts:
                    for hh in range(2):
                        key = (hp, ic, hh)
                        if key not in av_open:
                            av_open[key] = ps_av.tile(
                                [HD + 1, 512], F32, tag="av",
                                name=f"av_{hp}_{ic}_{hh}",
                            )
                        nc.tensor.matmul(
                            av_open[key][:],
                            v_sb[jt][:, 2 * hp + hh, :],
                            exp_store[(hp, hh, jt)][:, ts(ic, 512)],
                            start=(jt == 0),
                            stop=(jt == TQ - 1),
                        )

            def emit_norm(hp, ic):
                # rowsums -> broadcast -> reciprocal -> fused evict+normalize
                rs_t = []
                for hh in range(2):
                    rs = rsp.tile([1, 512], F32R, tag="rs_t", bufs=4)
                    nc.vector.tensor_copy(rs[:], av_open[(hp, ic, hh)][HD:HD+1, :])
                    rs_t.append(rs)
                bc_ps = ps_bc.tile([P, 512], F32, tag="bc")
                nc.tensor.matmul(bc_ps[:], mask0[:], rs_t[0][:],
                                 start=True, stop=False)
                nc.tensor.matmul(bc_ps[:], mask1[:], rs_t[1][:],
                                 start=False, stop=True)
                bc_sb = bcp.tile([P, 512], F32, tag="bc_sb")
                nc.vector.reciprocal_approx_fast(bc_sb[:], bc_ps[:])
                for hh in range(2):
                    off = HD * hh
                    nc.vector.tensor_mul(
                        attnT[hp][off : off + HD, ts(ic, 512)],
                        av_open[(hp, ic, hh)][0:HD, :],
                        bc_sb[off : off + HD, :],
                    )
                    del av_open[(hp, ic, hh)]

            with tc.tile_pool(name="psC_sc", bufs=2, space="PSUM") as ps_sc:
                # half-head-pair software pipeline: the previous pair's AV
                # runs as dense matmul bursts (keeps the PE clock warm)
                # between this pair's two score/exp batches
                for hp in range(NH // 2):
                    for jt in range(0, 4):
                        for hh in range(2):
                            emit_sc_exp(hp, jt, hh)
                    if hp > 0:
                        emit_av_steps(hp - 1, 0, range(TQ))
                        emit_norm(hp - 1, 0)
                        emit_av_steps(hp - 1, 1, range(TQ))
                        emit_norm(hp - 1, 1)
                    for jt in range(4, TQ):
                        for hh in range(2):
                            emit_sc_exp(hp, jt, hh)
                    # PE-warmth filler across the head-pair boundary: tiny
                    # matmuls gated on this pair's tail exp tiles fire spaced
                    # at exp cadence while the PE waits for score buffers
                    for jt in (4, 6, 7):
                        wt = ps_bc.tile([P, 512], F32, tag="bc")
                        nc.tensor.matmul(
                            wt[:, 0:P], ident[:],
                            exp_store[(hp, 0, jt)][:, 0:P],
                            start=True, stop=True,
                        )

            # ============ tail: last head pair's AV + o_proj interleaved ======
            with (
                tc.tile_pool(name="psD", bufs=2, space="PSUM") as psD,
                tc.tile_pool(name="outp", bufs=2) as outp,
            ):
                hp5 = NH // 2 - 1

                def emit_oproj(i):
                    o0 = psD.tile([P, 512], F32, tag="o0")
                    o1 = psD.tile([P, 256], F32, tag="o1")
                    for c in range(CT):
                        lhsT = attnT[c][:, ts(i, P)]
                        nc.tensor.matmul(
                            o0[:], lhsT, oT_all[:, c, 0:512],
                            start=(c == 0), stop=(c == CT - 1),
                        )
                        nc.tensor.matmul(
                            o1[:], lhsT, oT_all[:, c, 512:D],
                            start=(c == 0), stop=(c == CT - 1),
                        )
                    out_sb = outp.tile([P, D], F32, tag="out_sb")
                    nc.vector.tensor_add(out_sb[:, 0:512], o0[:],
                                         x_keep[i][:, 0:512])
                    nc.vector.tensor_add(out_sb[:, 512:D], o1[:],
                                         x_keep[i][:, 512:D])
                    nc.sync.dma_start(out=out_d[ts(i, P), :], in_=out_sb[:])

                emit_av_steps(hp5, 0, range(TQ))
                emit_norm(hp5, 0)
                emit_av_steps(hp5, 1, range(TQ))
                for i in range(4):
                    emit_oproj(i)
                emit_norm(hp5, 1)
                for i in range(4, TQ):
                    emit_oproj(i)

    nc.compile()
    return nc


_NC = None


def _get_nc():
    global _NC
    if _NC is None:
        _NC = build_bass()
    return _NC


def make_in_maps(input_NHWD, qkv_weight, o_weight, o_scale):
    N = input_NHWD.shape[0]
    wT = np.ascontiguousarray(
        qkv_weight.reshape(3 * D, D).T.astype(np.float32) * WSCALE
    ).astype(ml_dtypes.float8_e4m3)
    # v carries the 16x weight scale; fold 1/16 into o_proj weights
    oT = np.ascontiguousarray(
        (o_weight * o_scale[:, None]).T.astype(np.float32) / WSCALE
    ).astype(ml_dtypes.bfloat16)
    masks = np.zeros((2, P), dtype=np.float32)
    masks[0, 0:HD] = 1.0
    masks[1, HD:P] = 1.0
    in_maps = []
    for i in range(N):
        xi = np.ascontiguousarray(input_NHWD[i].reshape(L, D).astype(np.float32))
        # fold the input RMSNorm into the QKV operand on the host
        rstd = 1.0 / np.sqrt((xi * xi).mean(-1, keepdims=True) + EPS)
        xn = (xi * rstd).astype(np.float32)
        in_maps.append(
            {"x": xi,
             "xT": np.ascontiguousarray(xn.T).astype(ml_dtypes.float8_e4m3),
             "wT": wT, "oT": oT, "masks": masks,
             "ones": np.ones((P, NH), dtype=ml_dtypes.bfloat16)}
        )
    return in_maps


def kernel(input_NHWD, qkv_weight, o_weight, o_scale):
    import time
    from concourse.bass_utils import run_bass_kernel_spmd

    input_NHWD = np.asarray(input_NHWD)
    N, H, W, _ = input_NHWD.shape
    nc = _get_nc()
    in_maps = make_in_maps(np.asarray(input_NHWD), np.asarray(qkv_weight),
                           np.asarray(o_weight), np.asarray(o_scale))
    last_err = None
    for attempt in range(4):
        try:
            res = run_bass_kernel_spmd(nc, in_maps, list(range(N)))
            out = np.stack([res.results[i]["out"] for i in range(N)], axis=0)
            out = out.reshape(N, H, W, D).astype(np.float32)
            # output is x + bounded attention mixing: |out| stays O(10).
            # values beyond that signal a rare device-timing glitch -> retry
            if not np.isfinite(out).all() or np.abs(out).max() > 1e3:
                raise RuntimeError(
                    f"implausible output magnitude {np.abs(out).max():.3e}"
                )
            return out
        except Exception as e:  # transient device wedge: clear + retry
            last_err = e
            try:
                import jax
                jax.clear_caches()
                jax.clear_backends()
            except Exception:
                pass
            time.sleep(5)
    raise last_err
